# revision 1
# baseline (speedup 1.0000x reference)
"""DANet (dual attention) Trainium2 kernel, v2.

Problem shapes (hardcoded): x [4, 64, 64, 64] f32, O = 16, N = H*W = 4096.
Sharding: 8 cores = 4 batches x 2 query-row halves (2048 query rows each).

v2 design notes (vs v1 baseline):
 - every heavy matmul is 1024-col moving-operand bf16 (the max), halving
   instruction count and sync hops: 64 ST + 64 attv MMs + 64 exps total.
 - per-core inputs are fed ROTATED (query half first) so the program is
   SPMD-uniform with no per-core constants; softmax over keys is
   permutation-invariant so only the query column order matters.
 - PSUM: tag "st" 2x[128,1024] (4 banks) + tag "gen" 2x[65,1024]
   (4 banks) = all 8 banks. attv accumulates 8-chunk bursts in a gen
   tile, drained by DVE adds into an SBUF accumulator; gen also serves
   projections / channel energy / transpose / oc / the rbp broadcast.
 - denominator rides as vt column 0 (acc row 0); combine uses
   reciprocal_approx_fast (1 DVE op) + ones-matmul broadcast consumed
   straight from PSUM.
 - channel attention: xt f32 energy burst mid-pass-0; gamma_ca*I and 2I
   come in via the eyepack input (no recompile on gamma change);
   oc = (gamma_ca*att_c^T + 2I)^T @ xq1 precomputed into SBUF.
"""

import sys

for _p in ("/opt/trn_rl_repo",):
    if _p not in sys.path:
        sys.path.append(_p)

import numpy as np
import ml_dtypes
from contextlib import ExitStack

import concourse.bass as bass
import concourse.bacc as bacc
import concourse.mybir as mybir
import concourse.tile as tile

F32 = mybir.dt.float32
BF16 = mybir.dt.bfloat16
AF = mybir.ActivationFunctionType
AX = mybir.AxisListType
ALU = mybir.AluOpType

B, C, H, W = 4, 64, 64, 64
N = H * W          # 4096
O = C // 4         # 16
NQ = N // 2        # 2048 query rows per core
NK_CH = N // 128   # 32 key chunks of 128
U = C + 1          # 65: ones column + 64 v channels


def build_program(gamma_ca: float = 0.0, repeat: int = 1, loop_n: int = 0,
                  debug_dump: bool = False):
    nc = bacc.Bacc(
        "TRN2", target_bir_lowering=False, debug=False, num_devices=8
    )

    xf1h_d = nc.dram_tensor("xf1h", [C + 1, N], BF16, kind="ExternalInput").ap()
    xq1_d = nc.dram_tensor("xq1", [C, NQ], F32, kind="ExternalInput").ap()
    xt_d = nc.dram_tensor("xt", [128, NK_CH * C], F32,
                          kind="ExternalInput").ap()
    wpack_d = nc.dram_tensor("wpack", [C + 1, 2 * O + C], BF16,
                             kind="ExternalInput").ap()
    eyepack_d = nc.dram_tensor("eyepack", [C, 2 * C], F32,
                               kind="ExternalInput").ap()
    out_d = nc.dram_tensor("out", [C, NQ], F32, kind="ExternalOutput").ap()
    dbg = None
    if debug_dump:
        dbg = {
            "accsb": nc.dram_tensor("d_accsb", [U, NQ], F32,
                                    kind="ExternalOutput").ap(),
            "ocsb": nc.dram_tensor("d_ocsb", [C, NQ], F32,
                                   kind="ExternalOutput").ap(),
            "rec": nc.dram_tensor("d_rec", [2, 1024], F32,
                                  kind="ExternalOutput").ap(),
            "rbp": nc.dram_tensor("d_rbp", [C, NQ], F32,
                                  kind="ExternalOutput").ap(),
            "attF": nc.dram_tensor("d_attF", [C, C], F32,
                                   kind="ExternalOutput").ap(),
        }

    with tile.TileContext(nc) as tc:
        with ExitStack() as ctx:
            consts = ctx.enter_context(tc.tile_pool(name="consts", bufs=2))
            expp = ctx.enter_context(tc.tile_pool(name="expp", bufs=4))
            sm = ctx.enter_context(tc.tile_pool(name="sm", bufs=2))
            outp = ctx.enter_context(tc.tile_pool(name="outp", bufs=2))
            pst = ctx.enter_context(
                tc.tile_pool(name="pst", bufs=2, space="PSUM"))
            pgen = ctx.enter_context(
                tc.tile_pool(name="pgen", bufs=2, space="PSUM"))

            warm = sm.tile([1, 16], F32, tag="warm")
            nc.vector.memset(warm[:], 0.0)
            nc.scalar.activation(warm[:], warm[:], AF.Exp)

            def emit_all():
                for _rep in range(repeat):
                    _emit_body(tc, consts, expp, sm, outp, pst, pgen,
                               xf1h_d, xq1_d, xt_d, wpack_d, eyepack_d,
                               out_d, dbg=dbg)

            if loop_n:
                with tc.For_i(
                    0, loop_n, 1,
                    hint_engines=(mybir.EngineType.PE,
                                  mybir.EngineType.Activation),
                ):
                    emit_all()
            else:
                emit_all()
    nc.compile()
    return nc


def _emit_body(tc, consts, expp, sm, outp, pst, pgen,
               xf1h_d, xq1_d, xt_d, wpack_d, eyepack_d, out_d, dbg=None):
    nc = tc.nc

    # ---- input DMAs (queue round-robin: earliest-needed first) ----
    wpack = consts.tile([C + 1, 2 * O + C], BF16, tag="wpack")
    nc.sync.dma_start(wpack[:], wpack_d[:])
    xf1h = consts.tile([C + 1, N], BF16, tag="xf1h")
    for j in range(2):
        nc.sync.dma_start(
            xf1h[:, j * 2048:(j + 1) * 2048],
            xf1h_d[:, j * 2048:(j + 1) * 2048])
    # xt sbuf layout: [128, 32*64]; chunk i at cols i*64:(i+1)*64 holds
    # xT rows i*128:(i+1)*128. Host pre-arranges so the DMA is contiguous.
    xt = consts.tile([128, NK_CH * C], F32, tag="xt")
    for g in range(2):
        nc.sync.dma_start(xt[:, g * 1024:(g + 1) * 1024],
                          xt_d[:, g * 1024:(g + 1) * 1024])
    xq1 = consts.tile([C, NQ], F32, tag="xq1")
    nc.sync.dma_start(xq1[:], xq1_d[:])
    eyepack = consts.tile([C, 2 * C], F32, tag="eyepack")
    nc.sync.dma_start(eyepack[:], eyepack_d[:])

    ones65 = consts.tile([U, C], F32, tag="ones65")
    nc.vector.memset(ones65[:], 1.0)
    sel64 = consts.tile([U, 1], F32, tag="sel64")
    nc.vector.memset(sel64[:], 0.0)
    nc.vector.memset(sel64[C:U, :], 1.0)

    # ---- q/k projections: 1024-wide bf16 matmuls via gen-tag psum ----
    k_sb = consts.tile([O, N], BF16, tag="k")
    for j in range(4):
        pk = pgen.tile([O, 1024], F32, tag="gen", name=f"pk{j}")
        for j2 in range(2):
            nc.tensor.matmul(
                pk[:, j2 * 512:(j2 + 1) * 512], wpack[:, O:2 * O],
                xf1h[:, j * 1024 + j2 * 512:j * 1024 + (j2 + 1) * 512],
                start=True, stop=True)
        if j % 2 == 0:
            nc.vector.tensor_copy(k_sb[:, j * 1024:(j + 1) * 1024], pk[:])
        else:
            nc.scalar.copy(k_sb[:, j * 1024:(j + 1) * 1024], pk[:])
    q_sb = consts.tile([O, NQ], BF16, tag="q")
    for j in range(2):
        pq = pgen.tile([O, 1024], F32, tag="gen", name=f"pq{j}")
        for j2 in range(2):
            nc.tensor.matmul(
                pq[:, j2 * 512:(j2 + 1) * 512], wpack[:, 0:O],
                xf1h[:, j * 1024 + j2 * 512:j * 1024 + (j2 + 1) * 512],
                start=True, stop=True)
        if j % 2 == 0:
            nc.vector.tensor_copy(q_sb[:, j * 1024:(j + 1) * 1024], pq[:])
        else:
            nc.scalar.copy(q_sb[:, j * 1024:(j + 1) * 1024], pq[:])

    # ---- v^T chunks with a trailing ones column (denominator row 64) ----
    vt = consts.tile([128, NK_CH * U], BF16, tag="vt")
    vt3 = vt[:].rearrange("p (c u) -> p c u", u=U)
    nc.vector.memset(vt3[:, :, C:U], 1.0)
    for g in range(8):
        pv = pgen.tile([128, 4 * C], F32, tag="gen", name=f"pv{g}")
        for q4 in range(4):
            i = g * 4 + q4
            nc.tensor.matmul(pv[:, q4 * C:(q4 + 1) * C],
                             xf1h[:, i * 128:(i + 1) * 128],
                             wpack[:, 2 * O:2 * O + C],
                             start=True, stop=True)
        dst = vt3[:, g * 4:(g + 1) * 4, 0:C]
        src = pv[:].rearrange("p (c f) -> p c f", f=C)
        if g % 2 == 0:
            nc.vector.tensor_copy(dst, src)
        else:
            nc.scalar.copy(dst, src)

    # ---- state for the main stream ----
    accsb = sm.tile([U, NQ], F32, tag="accsb")
    ocsb = sm.tile([C, NQ], F32, tag="ocsb")
    attF = sm.tile([C, C], F32, tag="attF")
    ex_tiles = {}
    burst = {}
    chan = {}

    def emit_st(idx):
        p, i = idx // NK_CH, idx % NK_CH
        st = pst.tile([128, 1024], F32, tag="st", name=f"st{p}_{i}")
        for j2 in range(2):
            nc.tensor.matmul(
                st[:, j2 * 512:(j2 + 1) * 512],
                k_sb[:, i * 128:(i + 1) * 128],
                q_sb[:, p * 1024 + j2 * 512:p * 1024 + (j2 + 1) * 512],
                start=True, stop=True)
        ex = expp.tile([128, 1024], BF16, tag="ex", name=f"ex{p}_{i}")
        nc.scalar.activation(ex[:], st[:], AF.Exp)
        ex_tiles[idx] = ex

    def emit_attv(p, i):
        ex = ex_tiles.pop(p * NK_CH + i)
        if i % 8 == 0:
            burst["t"] = pgen.tile([U, 1024], F32, tag="gen",
                                   name=f"acc{p}_{i // 8}")
        for j2 in range(2):
            nc.tensor.matmul(burst["t"][:, j2 * 512:(j2 + 1) * 512],
                             vt[:, i * U:(i + 1) * U],
                             ex[:, j2 * 512:(j2 + 1) * 512],
                             start=(i % 8 == 0), stop=(i % 8 == 7))
        if i % 8 == 7:
            ps = slice(p * 1024, (p + 1) * 1024)
            if i == 7:
                nc.vector.tensor_copy(accsb[:, ps], burst["t"][:])
            else:
                nc.vector.tensor_add(accsb[:, ps], accsb[:, ps],
                                     burst["t"][:])

    def emit_energy():
        en = pgen.tile([C, C], F32, tag="gen", name="en")
        for i in range(NK_CH):
            nc.tensor.matmul(en[:], xt[:, i * C:(i + 1) * C],
                             xt[:, i * C:(i + 1) * C],
                             start=(i == 0), stop=(i == NK_CH - 1))
        chan["en"] = en

    def emit_channel_softmax():
        en = chan.pop("en")
        emin = sm.tile([C, 1], F32, tag="emin")
        nc.vector.tensor_reduce(emin[:], en[:], axis=AX.X, op=ALU.min)
        ae = sm.tile([C, C], F32, tag="ae")
        esum = sm.tile([C, 1], F32, tag="esum")
        nc.scalar.activation(ae[:], en[:], AF.Exp, bias=emin[:], scale=-1.0,
                             accum_out=esum[:])
        esr = sm.tile([C, 1], F32, tag="esr")
        nc.vector.reciprocal(esr[:], esum[:])
        ac = sm.tile([C, C], F32, tag="ac")
        nc.vector.tensor_scalar_mul(ac[:], ae[:], esr[:])
        at = pgen.tile([C, C], F32, tag="gen", name="at")
        nc.tensor.matmul(at[:], ac[:], eyepack[:, 0:C], start=True, stop=True)
        nc.vector.tensor_add(attF[:], at[:], eyepack[:, C:2 * C])

    def emit_oc(half):
        po = pgen.tile([C, 1024], F32, tag="gen", name=f"oc{half}")
        for j2 in range(2):
            nc.tensor.matmul(
                po[:, j2 * 512:(j2 + 1) * 512], attF[:],
                xq1[:, half * 1024 + j2 * 512:half * 1024 + (j2 + 1) * 512],
                start=True, stop=True)
        nc.vector.tensor_copy(ocsb[:, half * 1024:(half + 1) * 1024], po[:])

    def emit_combine(p):
        ps = slice(p * 1024, (p + 1) * 1024)
        # move the denominator row (partition 64) to partition 0 via a
        # selector matmul: the custom-DVE reciprocal misbehaves on HW at
        # partition offsets != 0.
        pd = pgen.tile([1, 1024], F32, tag="gen", name=f"pd{p}")
        for j2 in range(2):
            nc.tensor.matmul(
                pd[:, j2 * 512:(j2 + 1) * 512], sel64[:],
                accsb[:, p * 1024 + j2 * 512:p * 1024 + (j2 + 1) * 512],
                start=True, stop=True)
        rec = sm.tile([1, 1024], F32, tag="rec", name=f"rec{p}", bufs=1)
        nc.vector.reciprocal_approx_fast(rec[:], pd[:])
        rbp = pgen.tile([C, 1024], F32, tag="gen", name=f"rbp{p}")
        for j2 in range(2):
            nc.tensor.matmul(rbp[:, j2 * 512:(j2 + 1) * 512],
                             ones65[0:1, :],
                             rec[:, j2 * 512:(j2 + 1) * 512],
                             start=True, stop=True)
        t1 = sm.tile([C, 1024], F32, tag="t1", name=f"t1{p}", bufs=1)
        ob = outp.tile([C, 1024], F32, tag="ob", name=f"ob{p}")
        for j2 in range(2):
            hs = slice(j2 * 512, (j2 + 1) * 512)
            gs = slice(p * 1024 + j2 * 512, p * 1024 + (j2 + 1) * 512)
            nc.vector.tensor_mul(t1[:, hs], accsb[0:C, gs], rbp[:, hs])
            nc.vector.tensor_add(ob[:, hs], t1[:, hs], ocsb[:, gs])
            nc.sync.dma_start(out_d[:, gs], ob[:, hs])
        if dbg is not None:
            nc.sync.dma_start(dbg["accsb"][:, ps], accsb[:, ps])
            nc.sync.dma_start(dbg["ocsb"][:, ps], ocsb[:, ps])
            nc.sync.dma_start(dbg["rec"][p:p + 1, :], rec[:])
            rbps = sm.tile([C, 1024], F32, tag="rbps", name=f"rbps{p}")
            nc.vector.tensor_copy(rbps[:], rbp[:])
            nc.sync.dma_start(dbg["rbp"][:, ps], rbps[:])
            if p == 0:
                nc.sync.dma_start(dbg["attF"][:], attF[:])

    # ---- main stream: skew-2 software pipeline over 64 chunks ----
    emit_st(0)
    emit_st(1)
    for idx in range(2 * NK_CH):
        p, i = idx // NK_CH, idx % NK_CH
        if idx + 2 < 2 * NK_CH:
            emit_st(idx + 2)
        emit_attv(p, i)
        if p == 0:
            if i == 8:
                emit_energy()
            elif i == 9:
                emit_channel_softmax()
            elif i == 11:
                emit_oc(0)
            elif i == 13:
                emit_oc(1)
        if i == NK_CH - 1:
            emit_combine(p)


# ---------------- host side ----------------

_PROGRAM_CACHE = {}


def _get_program(gamma_ca: float = 0.0):
    # gamma_ca rides in via the eyepack input; one program serves all.
    if "p" not in _PROGRAM_CACHE:
        _PROGRAM_CACHE["p"] = build_program()
    return _PROGRAM_CACHE["p"]


def build_in_maps(x, wq, bq, wk, bk, wv, bv, gamma_ca, gamma_sa):
    bf16 = np.dtype(ml_dtypes.bfloat16)
    x = np.asarray(x, dtype=np.float32)
    wq = np.asarray(wq, dtype=np.float32)
    bq = np.asarray(bq, dtype=np.float32)
    wk = np.asarray(wk, dtype=np.float32)
    bk = np.asarray(bk, dtype=np.float32)
    wv = np.asarray(wv, dtype=np.float32)
    bv = np.asarray(bv, dtype=np.float32)
    g_ca = float(np.asarray(gamma_ca).reshape(-1)[0])
    g_sa = float(np.asarray(gamma_sa).reshape(-1)[0])

    xf = x.reshape(B, C, N)
    xt_pre = [
        np.ascontiguousarray(
            xf[b].T.reshape(NK_CH, 128, C).transpose(1, 0, 2).reshape(
                128, NK_CH * C))
        for b in range(B)
    ]
    ones_row = np.ones((1, N), np.float32)
    qT1 = np.concatenate([wq.T, bq[None, :]], axis=0)
    kT1 = np.concatenate([wk.T, bk[None, :]], axis=0)
    wvT1 = g_sa * np.concatenate([wv.T, bv[None, :]], axis=0)
    wpack = np.ascontiguousarray(
        np.concatenate([qT1, kT1, wvT1], axis=1).astype(bf16))
    eye = np.eye(C, dtype=np.float32)
    eyepack = np.ascontiguousarray(
        np.concatenate([g_ca * eye, 2.0 * eye], axis=1))

    in_maps = []
    for core in range(8):
        b, h = core // 2, core % 2
        # rotate: this core's query half first (softmax over keys is
        # permutation-invariant; only query column order matters)
        xrot = np.concatenate(
            [xf[b][:, h * NQ:(h + 1) * NQ], xf[b][:, (1 - h) * NQ:(2 - h) * NQ]],
            axis=1)
        xf1 = np.concatenate([xrot, ones_row], axis=0)
        in_maps.append({
            "xf1h": np.ascontiguousarray(xf1.astype(bf16)),
            "xq1": np.ascontiguousarray(xrot[:, 0:NQ]),
            "xt": xt_pre[b],
            "wpack": wpack,
            "eyepack": eyepack,
        })
    return in_maps


LAST_RESULTS = None


def kernel(x, wq, bq, wk, bk, wv, bv, gamma_ca, gamma_sa):
    global LAST_RESULTS
    from concourse.bass_utils import run_bass_kernel_spmd

    nc = _get_program()
    in_maps = build_in_maps(x, wq, bq, wk, bk, wv, bv, gamma_ca, gamma_sa)

    res = run_bass_kernel_spmd(nc, in_maps, list(range(8)))
    LAST_RESULTS = res
    out = np.empty((B, C, N), np.float32)
    for core in range(8):
        b, h = core // 2, core % 2
        out[b, :, h * NQ:(h + 1) * NQ] = res.results[core]["out"]
    return out.reshape(B, C, H, W)

